# revision 11
# baseline (speedup 1.0000x reference)
"""Trainium2 Bass kernel for nn_ALBertMultiheadAttention (Lorentz/hyperbolic MHA).

Head-sharded tensor parallel across 8 NeuronCores (2 of 16 heads per core).
v2 design:
- QKV projections feature-major (bf16), RoPE via pair-swap matmul, Lorentz
  time-lift from the PRE-rope sum of squares (rotation invariance) with q+k
  lifts batched into one col-tiled PSUM tile per block (one Ln+Exp pair).
- Attention computed transposed; the two local heads are packed onto the PE
  array concurrently: score matmuls row-tiled (K=64 each, tile_position
  (0,0)/(64,0)), centroid matmuls col-tiled ((0,0)/(0,64)) into one [128,1024]
  accumulator. Softmax denominator cancels in the Lorentz renormalization.
- exp() split across engines: ACT computes exact Exp for most key-tiles, DVE
  computes a Schraudolph bit-trick exp (int16 bits of bf16) for the rest,
  so the two engines stream score tiles concurrently.
- Per-batch AllToAll (4 small collectives) overlapped with the next batch's
  attention; output projection pipelined per batch.
"""

import sys

sys.path.insert(0, "/opt/trn_rl_repo")

from contextlib import ExitStack

import numpy as np

B, S, HID = 4, 2048, 1024
H, HD = 16, 64
NCORES = 8
NT = B * S
NB = 4      # 512-token projection blocks per batch
BLK = 512
QC = 1024   # attention query-chunk width
NKT = S // 128  # 16 key tiles per batch

# Schraudolph exp: bits_bf16(exp(s)) ~= A*s + B_ (s = raw score, scale folded)
SCALE = float(HD ** -0.5)
SCHR_A = 128.0 * 1.4426950408889634 * SCALE
SCHR_B = 127.0 * 128.0 - 4.6

# (h, kt) steps whose exp runs on DVE (Schraudolph); rest on ACT (exact).
DVE_STEP = lambda h, kt: h == 1 and kt % 2 == 0

_GRAPH_CACHE = {}


def _host_prep(hidden_states, Wq, bq, Wk, bk, Wv, bv, Wc, bc, cos, sin, c, rope_dim):
    rd = int(np.asarray(rope_dim))
    cc = float(np.asarray(c).reshape(-1)[0])
    f32 = np.float32

    import ml_dtypes
    bf16 = ml_dtypes.bfloat16
    hT = np.ascontiguousarray(hidden_states.reshape(NT, HID).T.astype(bf16))

    # Interleaved-table usage: ce[2i] = ce[2i+1] = cos[2i]
    cos_eff = np.repeat(np.asarray(cos, f32)[:, 0:rd:2], 2, axis=1)  # [S, rd]
    sin_eff = np.repeat(np.asarray(sin, f32)[:, 0:rd:2], 2, axis=1)
    cosA = np.zeros((128, S), f32)
    sinA = np.zeros((128, S), f32)
    for hh in (0, 1):
        base = 64 * hh + 1
        cosA[base:base + rd] = cos_eff.T
        cosA[base + rd:64 * hh + 64] = 1.0
        sinA[base:base + rd] = sin_eff.T

    # Pair-swap permutation as matmul lhsT (out = P^T @ x)
    P = np.zeros((128, 128), f32)
    for hh in (0, 1):
        base = 64 * hh + 1
        for i in range(rd // 2):
            P[base + 2 * i + 1, base + 2 * i] = -1.0
            P[base + 2 * i, base + 2 * i + 1] = 1.0

    ident = np.eye(128, dtype=f32)

    ones2 = np.zeros((128, 2), f32)   # lift: sum space rows per head
    ones2[1:64, 0] = 1.0
    ones2[65:128, 1] = 1.0
    sgn2 = np.zeros((128, 2), f32)    # renorm: Lorentz signature per head
    sgn2[0:64, 0] = 1.0
    sgn2[0, 0] = -1.0
    sgn2[64:128, 1] = 1.0
    sgn2[64, 1] = -1.0
    sel2 = np.zeros((2, 128), f32)    # broadcast rsqrt row back over 64 dims
    sel2[0, 0:64] = np.sqrt(cc)
    sel2[1, 64:128] = np.sqrt(cc)

    def aug_w(W, b, h0, h1):
        Wa = np.zeros((HID, 128), f32)
        ba = np.zeros((128, 1), f32)
        Wa[:, 1:64] = W[:, 63 * h0:63 * h0 + 63]
        Wa[:, 65:128] = W[:, 63 * h1:63 * h1 + 63]
        ba[1:64, 0] = b[63 * h0:63 * h0 + 63]
        ba[65:128, 0] = b[63 * h1:63 * h1 + 63]
        return Wa, ba

    wc_np = np.ascontiguousarray(np.asarray(Wc, f32).astype(bf16))
    bc_np = np.asarray(bc, f32).reshape(1, HID - 1)

    in_maps = []
    for r in range(NCORES):
        h0, h1 = 2 * r, 2 * r + 1
        wqa, bqa = aug_w(np.asarray(Wq, f32), np.asarray(bq, f32), h0, h1)
        wka, bka = aug_w(np.asarray(Wk, f32), np.asarray(bk, f32), h0, h1)
        wva, bva = aug_w(np.asarray(Wv, f32), np.asarray(bv, f32), h0, h1)
        in_maps.append({
            "hT": hT,
            "wq": wqa.astype(bf16), "wk": wka.astype(bf16), "wv": wva.astype(bf16),
            "bq": bqa, "bk": bka, "bv": bva,
            "wc": wc_np, "bc": bc_np,
            "cosA": cosA.astype(bf16), "sinA": sinA,
            "pswap": P.astype(bf16), "ident": ident.astype(bf16),
            "ones2": ones2.astype(bf16), "sgn2": sgn2.astype(bf16),
            "sel2": sel2,
        })
    return in_maps, cc


def _build(cc):
    import concourse.tile as tile
    from concourse import bacc, mybir
    from concourse.hw_specs import get_activation_tables as _orig_tables

    AFt = mybir.ActivationFunctionType
    _mine = {AFt.Exp, AFt.Ln, AFt.Copy, AFt.Identity, AFt.Square}

    def _pinned_tables(arch):
        out = {}
        for name, funcs in _orig_tables(arch).items():
            if name == "natural_log_exp_and_others":
                out[name] = funcs
            else:
                out[name] = funcs - _mine
        return out

    F32 = mybir.dt.float32
    BF16 = mybir.dt.bfloat16
    I16 = mybir.dt.int16
    AF = mybir.ActivationFunctionType
    ALU = mybir.AluOpType
    AX = mybir.AxisListType.X

    bacc.get_activation_tables = _pinned_tables
    nc = bacc.Bacc("TRN2", target_bir_lowering=False, debug=False,
                   num_devices=NCORES)

    d_hT = nc.dram_tensor("hT", [HID, NT], BF16, kind="ExternalInput")
    d_w = {k: nc.dram_tensor(k, [HID, 128], BF16, kind="ExternalInput")
           for k in ("wq", "wk", "wv")}
    d_b = {k: nc.dram_tensor(k, [128, 1], F32, kind="ExternalInput")
           for k in ("bq", "bk", "bv")}
    d_wc = nc.dram_tensor("wc", [HID, HID - 1], BF16, kind="ExternalInput")
    d_bc = nc.dram_tensor("bc", [1, HID - 1], F32, kind="ExternalInput")
    d_cos = nc.dram_tensor("cosA", [128, S], BF16, kind="ExternalInput")
    d_sin = nc.dram_tensor("sinA", [128, S], F32, kind="ExternalInput")
    d_pswap = nc.dram_tensor("pswap", [128, 128], BF16, kind="ExternalInput")
    d_ident = nc.dram_tensor("ident", [128, 128], BF16, kind="ExternalInput")
    d_ones2 = nc.dram_tensor("ones2", [128, 2], BF16, kind="ExternalInput")
    d_sgn2 = nc.dram_tensor("sgn2", [128, 2], BF16, kind="ExternalInput")
    d_sel2 = nc.dram_tensor("sel2", [2, 128], F32, kind="ExternalInput")
    d_out = nc.dram_tensor("out", [NT // NCORES, HID], F32, kind="ExternalOutput")

    with TileCtx(nc, tile) as (tc, ctx):
        sb = lambda name, bufs: ctx.enter_context(tc.tile_pool(name=name, bufs=bufs))
        consts = sb("consts", 1)
        ht_pool = sb("ht", 8)
        slabs = sb("slabs", 2)
        scratch = sb("scratch", 2)
        pt_pool = sb("pt", 4)
        nrm_pool = sb("nrm", 2)
        outs_pool = sb("outs", 2)
        ats_pool = sb("ats", 2)
        pp = ctx.enter_context(tc.tile_pool(name="pp", bufs=2, space="PSUM"))
        sc_ps = ctx.enter_context(tc.tile_pool(name="sc_ps", bufs=2, space="PSUM"))
        u_ps = ctx.enter_context(tc.tile_pool(name="u_ps", bufs=1, space="PSUM"))
        dram = ctx.enter_context(tc.tile_pool(name="dram", bufs=1, space="DRAM"))

        # ---- constants ----
        w_sb = {}
        for k in ("wq", "wk", "wv"):
            t = consts.tile([128, HID], BF16, name=f"{k}_sb")
            for kk in range(8):
                nc.sync.dma_start(t[:, 128 * kk:128 * (kk + 1)],
                                  d_w[k][128 * kk:128 * (kk + 1), :])
            w_sb[k] = t
        b_sb = {}
        for k in ("bq", "bk", "bv"):
            t = consts.tile([128, 1], F32, name=f"{k}_sb")
            nc.sync.dma_start(t[:], d_b[k][:])
            b_sb[k] = t
        NO = HID - 1  # 1023
        wc_sb = consts.tile([128, 8 * NO], BF16, name="wc_sb")
        for kk in range(8):
            nc.sync.dma_start(wc_sb[:, NO * kk:NO * (kk + 1)],
                              d_wc[128 * kk:128 * (kk + 1), :])
        bc_sb = consts.tile([1, NO], F32, name="bc_sb")
        nc.sync.dma_start(bc_sb[:], d_bc[:])
        cos_sb = consts.tile([128, S], BF16, name="cos_sb")
        nc.sync.dma_start(cos_sb[:], d_cos[:])
        sin_sb = consts.tile([128, S], F32, name="sin_sb")
        nc.sync.dma_start(sin_sb[:], d_sin[:])
        pswap_sb = consts.tile([128, 128], BF16, name="pswap_sb")
        nc.sync.dma_start(pswap_sb[:], d_pswap[:])
        ident_sb = consts.tile([128, 128], BF16, name="ident_sb")
        nc.sync.dma_start(ident_sb[:], d_ident[:])
        ones2 = consts.tile([128, 2], BF16, name="ones2")
        nc.sync.dma_start(ones2[:], d_ones2[:])
        sgn2 = consts.tile([128, 2], BF16, name="sgn2")
        nc.sync.dma_start(sgn2[:], d_sgn2[:])
        sel2 = consts.tile([2, 128], F32, name="sel2")
        nc.sync.dma_start(sel2[:], d_sel2[:])
        onesrow = consts.tile([1, 128], F32, name="onesrow")
        nc.vector.memset(onesrow[:], 1.0)
        ccb = consts.tile([128, 1], F32, name="ccb")
        nc.vector.memset(ccb[:], cc)

        a2a_in = [dram.tile([NCORES, 128, 256], BF16, name=f"a2a_in{b}",
                            tag=f"a2a_in{b}") for b in range(B)]
        a2a_out = [dram.tile([NCORES, 128, 256], BF16, name=f"a2a_out{b}",
                             tag=f"a2a_out{b}") for b in range(B)]

        slab_of = {}  # b -> (qf, k, v)
        hts_of = {}   # b -> list of 8 [128, S] tiles

        def load_hts(b):
            hts = []
            for kk in range(8):
                htk = ht_pool.tile([128, S], BF16, name="htk", tag="ht")
                nc.sync.dma_start(htk[:], d_hT[128 * kk:128 * (kk + 1),
                                              S * b:S * (b + 1)])
                hts.append(htk)
            hts_of[b] = hts

        def proj_block(b, blk):
            """Projection + rope + lift for tokens [S*b + BLK*blk, +BLK)."""
            qf_slab, k_slab, v_slab = slab_of[b]
            cols = slice(BLK * blk, BLK * (blk + 1))
            hts = [t[:, cols] for t in hts_of[b]]

            ss = pp.tile([128, BLK], F32, name="ss_ps", tag="pp")
            for iq, (name, slab) in enumerate((("q", qf_slab), ("k", k_slab))):
                ps = pp.tile([128, BLK], F32, name="proj_ps", tag="pp")
                for kk in range(8):
                    nc.tensor.matmul(ps[:],
                                     w_sb["w" + name][:, 128 * kk:128 * (kk + 1)],
                                     hts[kk],
                                     start=(kk == 0), stop=(kk == 7))
                x_bf = scratch.tile([128, BLK], BF16, name="x_bf", tag="x")
                nc.vector.tensor_scalar_add(x_bf[:], ps[:], b_sb["b" + name][:])
                # lift sum-of-squares from PRE-rope x (rotation invariant);
                # q rows -> ss[0:2], k rows -> ss[32:34] (col-tiled)
                sq = scratch.tile([128, BLK], BF16, name="sq", tag="sq")
                nc.gpsimd.tensor_tensor(sq[:], x_bf[:], x_bf[:], ALU.mult)
                nc.tensor.matmul(ss[32 * iq:32 * iq + 2, :], ones2[:], sq[:],
                                 start=True, stop=True,
                                 tile_position=(0, 32 * iq))
                # rope
                swp = pp.tile([128, BLK], F32, name="swp_ps", tag="pp")
                nc.tensor.matmul(swp[:], pswap_sb[:], x_bf[:],
                                 start=True, stop=True)
                r1 = scratch.tile([128, BLK], BF16, name="r1", tag="r1")
                nc.vector.tensor_tensor(r1[:], x_bf[:], cos_sb[:, cols], ALU.mult)
                r2 = scratch.tile([128, BLK], BF16, name="r2", tag="r2")
                nc.vector.tensor_tensor(r2[:], swp[:], sin_sb[:, cols], ALU.mult)
                nc.vector.tensor_tensor(slab[:, cols], r1[:], r2[:], ALU.add)
            # q+k time rows: one Ln+Exp over the combined ss tile
            texp = scratch.tile([128, BLK], BF16, name="texp", tag="texp")
            lns = scratch.tile([128, BLK], F32, name="lns", tag="lns")
            nc.scalar.activation(lns[0:34, :], ss[0:34, :], AF.Ln, bias=ccb[0:34, :])
            nc.scalar.activation(texp[0:34, :], lns[0:34, :], AF.Exp, scale=0.5)
            tq = scratch.tile([2, BLK], BF16, name="tq", tag="tq")
            nc.vector.tensor_scalar_mul(tq[:], texp[0:2, :], -1.0)
            qrows = qf_slab[:].rearrange("(a c) n -> a c n", a=2)[:, 0, cols]
            nc.sync.dma_start(qrows, tq[:])
            krows = k_slab[:].rearrange("(a c) n -> a c n", a=2)[:, 0, cols]
            nc.sync.dma_start(krows, texp[32:34, :])

            # V: feature-major proj, transpose to token-major, lift post-hoc
            ps = pp.tile([128, BLK], F32, name="proj_ps", tag="pp")
            for kk in range(8):
                nc.tensor.matmul(ps[:],
                                 w_sb["wv"][:, 128 * kk:128 * (kk + 1)],
                                 hts[kk][:],
                                 start=(kk == 0), stop=(kk == 7))
            v_bf = scratch.tile([128, BLK], BF16, name="v_bf", tag="x")
            nc.vector.tensor_scalar_add(v_bf[:], ps[:], b_sb["bv"][:])
            tr = pp.tile([128, BLK], BF16, name="tr_ps", tag="pp")
            for j in range(4):
                nc.tensor.transpose(tr[:, 128 * j:128 * (j + 1)],
                                    v_bf[:, 128 * j:128 * (j + 1)], ident_sb[:])
            nc.vector.tensor_copy(v_slab[:, cols], tr[:])
            vview = v_slab[:, cols].rearrange("p (a h d) -> p a h d", a=4, h=2)
            sqv = scratch.tile([128, 4 * 2 * 63], F32, name="sqv", tag="sqv")
            sqvv = sqv[:].rearrange("p (a h d) -> p a h d", a=4, h=2)
            nc.vector.tensor_tensor(sqvv, vview[:, :, :, 1:64],
                                    vview[:, :, :, 1:64], ALU.mult)
            rv = scratch.tile([128, 8], F32, name="rv", tag="rv")
            nc.vector.tensor_reduce(rv[:].rearrange("p (a h) -> p a h", a=4),
                                    sqvv, AX, ALU.add)
            lnv = scratch.tile([128, 8], F32, name="lnv", tag="lnv")
            nc.scalar.activation(lnv[:], rv[:], AF.Ln, bias=ccb[:])
            tv = scratch.tile([128, 8], F32, name="tv", tag="tv")
            nc.scalar.activation(tv[:], lnv[:], AF.Exp, scale=0.5)
            nc.vector.tensor_copy(vview[:, :, :, 0:1],
                                  tv[:].rearrange("p (a h) -> p a h", a=4).unsqueeze(3))

        def attn_steps(b, qc, interleave):
            """32 (h, kt) steps + renorm + att store for query chunk qc."""
            qf_slab, k_slab, v_slab = slab_of[b]
            u = u_ps.tile([128, QC], F32, name="u_ps", tag="u")
            for s in range(32):
                if s in interleave:
                    interleave[s]()
                h = s % 2
                kt = s // 2
                hp = slice(64 * h, 64 * h + 64)
                sc = sc_ps.tile([128, QC], F32, name="sc_ps", tag="sc")
                for half in range(2):
                    nc.tensor.matmul(
                        sc[:, 512 * half:512 * (half + 1)],
                        k_slab[hp, 128 * kt:128 * (kt + 1)],
                        qf_slab[hp, QC * qc + 512 * half:QC * qc + 512 * (half + 1)],
                        start=True, stop=True, tile_position=(64 * h, 0))
                if DVE_STEP(h, kt):
                    pt_i = pt_pool.tile([128, QC], I16, name="pt_i", tag="pt_i")
                    nc.vector.tensor_scalar(pt_i[:], sc[:], SCHR_A, SCHR_B,
                                            ALU.mult, ALU.add)
                    pt = pt_i[:].bitcast(BF16)
                else:
                    pt_t = pt_pool.tile([128, QC], BF16, name="pt", tag="pt")
                    nc.scalar.activation(pt_t[:], sc[:], AF.Exp, scale=SCALE)
                    pt = pt_t[:]
                vt = v_slab[:, 128 * kt + 64 * h:128 * kt + 64 * h + 64]
                for half in range(2):
                    nc.tensor.matmul(
                        u[hp, 512 * half:512 * (half + 1)],
                        vt,
                        pt[:, 512 * half:512 * (half + 1)],
                        start=(kt == 0), stop=(kt == NKT - 1),
                        tile_position=(0, 64 * h))
            # Lorentz renormalize both heads at once (denominator cancels)
            ucp = nrm_pool.tile([128, QC], BF16, name="ucp", tag="ucp")
            nc.vector.tensor_copy(ucp[:], u[:])
            nsq = nrm_pool.tile([128, QC], BF16, name="nsq", tag="nsq")
            nc.vector.tensor_tensor(nsq[:], ucp[:], ucp[:], ALU.mult)
            for half in range(2):
                hs = slice(512 * half, 512 * (half + 1))
                lp = pp.tile([2, 512], F32, name="lp_ps", tag="pp")
                nc.tensor.matmul(lp[:], sgn2[:], nsq[:, hs], start=True, stop=True)
                lg = nrm_pool.tile([2, 512], F32, name="lg", tag="lg")
                nc.scalar.activation(lg[:], lp[:], AF.Ln, scale=-1.0)
                rr = nrm_pool.tile([2, 512], F32, name="rr", tag="rr")
                nc.scalar.activation(rr[:], lg[:], AF.Exp, scale=-0.5)
                rb = pp.tile([128, 512], F32, name="rb_ps", tag="pp")
                nc.tensor.matmul(rb[:], sel2[:], rr[:], start=True, stop=True)
                att = nrm_pool.tile([128, 512], BF16, name="att", tag="att")
                nc.vector.tensor_tensor(att[:], ucp[:, hs], rb[:], ALU.mult)
                for cx in range(2):
                    j = 4 * qc + 2 * half + cx
                    nc.sync.dma_start(a2a_in[b][j, :, :],
                                      att[:, 256 * cx:256 * (cx + 1)])

        def do_a2a(b):
            nc.gpsimd.collective_compute(
                "AllToAll", mybir.AluOpType.bypass,
                replica_groups=[list(range(NCORES))],
                ins=[a2a_in[b][:].opt()],
                outs=[a2a_out[b][:].opt()],
            )

        atts_of = {}

        def load_atts(b):
            t = ats_pool.tile([128, 8 * 256], BF16, name="attk", tag="attk")
            nc.sync.dma_start(t[:].rearrange("p (k c) -> p k c", k=8),
                              a2a_out[b][:].rearrange("k p c -> p k c"))
            atts_of[b] = t

        def outproj_m(b, m):
            """Output projection for 128 tokens (m-th tile of batch b's slice)."""
            att_k = atts_of[b]
            msl = slice(128 * m, 128 * (m + 1))
            pss = []
            for n0 in (0, NO - 512):  # second half overlaps one col
                po = pp.tile([128, 512], F32, name="out_ps", tag="pp")
                for kk in range(8):
                    nc.tensor.matmul(po[:],
                                     att_k[:, 256 * kk + msl.start:
                                           256 * kk + msl.stop],
                                     wc_sb[:, NO * kk + n0:NO * kk + n0 + 512],
                                     start=(kk == 0), stop=False)
                nc.tensor.matmul(po[:], onesrow[:], bc_sb[:, n0:n0 + 512],
                                 start=False, stop=True)
                pss.append(po)
            out_sb = outs_pool.tile([128, HID], F32, name="out_sb", tag="out")
            nc.vector.tensor_copy(out_sb[:, 1:513], pss[0][:])
            nc.vector.tensor_copy(out_sb[:, 513:HID], pss[1][:, 1:512])
            sqo = scratch.tile([128, NO], BF16, name="sqo", tag="sqo")
            nc.vector.tensor_tensor(sqo[:], out_sb[:, 1:HID], out_sb[:, 1:HID],
                                    ALU.mult)
            ro = scratch.tile([128, 1], F32, name="ro", tag="ro")
            nc.vector.tensor_reduce(ro[:], sqo[:].unsqueeze(1), AX, ALU.add)
            lno = scratch.tile([128, 1], F32, name="lno", tag="lno")
            nc.scalar.activation(lno[:], ro[:], AF.Ln, bias=ccb[:])
            nc.scalar.activation(out_sb[:, 0:1], lno[:], AF.Exp, scale=0.5)
            nc.sync.dma_start(d_out[256 * b + msl.start:256 * b + msl.stop, :],
                              out_sb[:])

        # ---------------- emission schedule ----------------
        def new_slabs(b):
            slab_of[b] = (
                slabs.tile([128, S], BF16, name="qf_slab", tag="qf"),
                slabs.tile([128, S], BF16, name="k_slab", tag="kk"),
                slabs.tile([128, S], BF16, name="v_slab", tag="vv"),
            )

        new_slabs(0)
        load_hts(0)
        for blk in range(NB):
            proj_block(0, blk)

        for b in range(B):
            if b > 0:
                do_a2a(b - 1)
                load_atts(b - 1)
            # qc=0: interleave next batch's projection blocks
            il0 = {}
            if b + 1 < B:
                new_slabs(b + 1)
                il0 = {0: lambda bb=b: load_hts(bb + 1),
                       2: lambda bb=b: proj_block(bb + 1, 0),
                       10: lambda bb=b: proj_block(bb + 1, 1),
                       18: lambda bb=b: proj_block(bb + 1, 2),
                       26: lambda bb=b: proj_block(bb + 1, 3)}
            attn_steps(b, 0, il0)
            # qc=1: interleave previous batch's output projection
            il1 = {}
            if b > 0:
                il1 = {8: lambda bb=b: outproj_m(bb - 1, 0),
                       20: lambda bb=b: outproj_m(bb - 1, 1)}
            attn_steps(b, 1, il1)

        do_a2a(B - 1)
        load_atts(B - 1)
        outproj_m(B - 1, 0)
        outproj_m(B - 1, 1)

    try:
        nc.compile()
    finally:
        bacc.get_activation_tables = _orig_tables
    return nc


class TileCtx:
    """TileContext + an ExitStack for pools, as one context manager."""

    def __init__(self, nc, tile_mod):
        self.nc = nc
        self.tile_mod = tile_mod

    def __enter__(self):
        self.ctx = ExitStack()
        self.tc = self.tile_mod.TileContext(self.nc)
        self.tc.__enter__()
        return self.tc, self.ctx

    def __exit__(self, *exc):
        self.ctx.close()
        return self.tc.__exit__(*exc)


def _get_graph(cc):
    key = round(cc, 9)
    if key not in _GRAPH_CACHE:
        _GRAPH_CACHE[key] = _build(cc)
    return _GRAPH_CACHE[key]


def _gather(results):
    """results[j]['out'] is [B*256, HID] covering, for each batch, tokens
    [qc*1024 + chunk*256, +256) with qc=j//4, chunk=j%4."""
    out = np.empty((B, S, HID), np.float32)
    for j in range(NCORES):
        r = np.asarray(results[j]["out"], np.float32).reshape(B, 256, HID)
        s0 = (j // 4) * 1024 + (j % 4) * 256
        out[:, s0:s0 + 256, :] = r
    return out


def run(trace=False, **inputs):
    from concourse.bass_utils import run_bass_kernel_spmd

    in_maps, cc = _host_prep(**inputs)
    nc = _get_graph(cc)
    res = run_bass_kernel_spmd(nc, in_maps, list(range(NCORES)), trace=trace)
    out = _gather(res.results)
    return out, res


def kernel(**inputs):
    out, _ = run(trace=False, **inputs)
    return out


# revision 13
# speedup vs baseline: 1.1871x; 1.1871x over previous
"""Trainium2 Bass kernel for nn_ALBertMultiheadAttention (Lorentz/hyperbolic MHA).

Head-sharded tensor parallel across 8 NeuronCores (2 of 16 heads per core).
v2 design:
- QKV projections feature-major (bf16), RoPE via pair-swap matmul, Lorentz
  time-lift from the PRE-rope sum of squares (rotation invariance) with q+k
  lifts batched into one col-tiled PSUM tile per block (one Ln+Exp pair).
- Attention computed transposed; the two local heads are packed onto the PE
  array concurrently: score matmuls row-tiled (K=64 each, tile_position
  (0,0)/(64,0)), centroid matmuls col-tiled ((0,0)/(0,64)) into one [128,1024]
  accumulator. Softmax denominator cancels in the Lorentz renormalization.
- exp() split across engines: ACT computes exact Exp for most key-tiles, DVE
  computes a Schraudolph bit-trick exp (int16 bits of bf16) for the rest,
  so the two engines stream score tiles concurrently.
- Per-batch AllToAll (4 small collectives) overlapped with the next batch's
  attention; output projection pipelined per batch.
"""

import sys

sys.path.insert(0, "/opt/trn_rl_repo")

from contextlib import ExitStack

import numpy as np

B, S, HID = 4, 2048, 1024
H, HD = 16, 64
NCORES = 8
NT = B * S
NB = 4      # 512-token projection blocks per batch
BLK = 512
QC = 1024   # attention query-chunk width
NKT = S // 128  # 16 key tiles per batch

# Schraudolph exp: bits_bf16(exp(s)) ~= A*s + B_ (s = raw score, scale folded)
SCALE = float(HD ** -0.5)
SCHR_A = 128.0 * 1.4426950408889634 * SCALE
SCHR_B = 127.0 * 128.0 - 4.6

# (h, kt) steps whose exp runs on DVE (Schraudolph); rest on ACT (exact).
DVE_STEP = lambda h, kt: h == 1 and kt % 2 == 0

_GRAPH_CACHE = {}


def _host_prep(hidden_states, Wq, bq, Wk, bk, Wv, bv, Wc, bc, cos, sin, c, rope_dim):
    rd = int(np.asarray(rope_dim))
    cc = float(np.asarray(c).reshape(-1)[0])
    f32 = np.float32

    import ml_dtypes
    bf16 = ml_dtypes.bfloat16
    hT = np.ascontiguousarray(hidden_states.reshape(NT, HID).T.astype(bf16))

    # Interleaved-table usage: ce[2i] = ce[2i+1] = cos[2i]
    cos_eff = np.repeat(np.asarray(cos, f32)[:, 0:rd:2], 2, axis=1)  # [S, rd]
    sin_eff = np.repeat(np.asarray(sin, f32)[:, 0:rd:2], 2, axis=1)
    cosA = np.zeros((128, S), f32)
    sinA = np.zeros((128, S), f32)
    for hh in (0, 1):
        base = 64 * hh + 1
        cosA[base:base + rd] = cos_eff.T
        cosA[base + rd:64 * hh + 64] = 1.0
        sinA[base:base + rd] = sin_eff.T

    # Pair-swap permutation as matmul lhsT (out = P^T @ x)
    P = np.zeros((128, 128), f32)
    for hh in (0, 1):
        base = 64 * hh + 1
        for i in range(rd // 2):
            P[base + 2 * i + 1, base + 2 * i] = -1.0
            P[base + 2 * i, base + 2 * i + 1] = 1.0

    ident = np.eye(128, dtype=f32)

    ones2 = np.zeros((128, 2), f32)   # lift: sum space rows per head
    ones2[1:64, 0] = 1.0
    ones2[65:128, 1] = 1.0
    sgn2 = np.zeros((128, 2), f32)    # renorm: Lorentz signature per head
    sgn2[0:64, 0] = 1.0
    sgn2[0, 0] = -1.0
    sgn2[64:128, 1] = 1.0
    sgn2[64, 1] = -1.0
    sel2 = np.zeros((2, 128), f32)    # broadcast rsqrt row back over 64 dims
    sel2[0, 0:64] = np.sqrt(cc)
    sel2[1, 64:128] = np.sqrt(cc)

    def aug_w(W, b, h0, h1):
        Wa = np.zeros((HID, 128), f32)
        ba = np.zeros((128, 1), f32)
        Wa[:, 1:64] = W[:, 63 * h0:63 * h0 + 63]
        Wa[:, 65:128] = W[:, 63 * h1:63 * h1 + 63]
        ba[1:64, 0] = b[63 * h0:63 * h0 + 63]
        ba[65:128, 0] = b[63 * h1:63 * h1 + 63]
        return Wa, ba

    wc_np = np.ascontiguousarray(np.asarray(Wc, f32).astype(bf16))
    bc_np = np.asarray(bc, f32).reshape(1, HID - 1)

    in_maps = []
    for r in range(NCORES):
        h0, h1 = 2 * r, 2 * r + 1
        wqa, bqa = aug_w(np.asarray(Wq, f32), np.asarray(bq, f32), h0, h1)
        wka, bka = aug_w(np.asarray(Wk, f32), np.asarray(bk, f32), h0, h1)
        wva, bva = aug_w(np.asarray(Wv, f32), np.asarray(bv, f32), h0, h1)
        in_maps.append({
            "hT": hT,
            "wq": wqa.astype(bf16), "wk": wka.astype(bf16), "wv": wva.astype(bf16),
            "bq": bqa, "bk": bka, "bv": bva,
            "wc": wc_np, "bc": bc_np,
            "cosA": cosA.astype(bf16), "sinA": sinA,
            "pswap": P.astype(bf16), "ident": ident.astype(bf16),
            "ones2": ones2.astype(bf16), "sgn2": sgn2.astype(bf16),
            "sel2": sel2,
        })
    return in_maps, cc


def _build(cc):
    import concourse.tile as tile
    from concourse import bacc, mybir
    from concourse.hw_specs import get_activation_tables as _orig_tables

    AFt = mybir.ActivationFunctionType
    _mine = {AFt.Exp, AFt.Ln, AFt.Copy, AFt.Identity, AFt.Square}

    def _pinned_tables(arch):
        out = {}
        for name, funcs in _orig_tables(arch).items():
            if name == "natural_log_exp_and_others":
                out[name] = funcs
            else:
                out[name] = funcs - _mine
        return out

    F32 = mybir.dt.float32
    BF16 = mybir.dt.bfloat16
    I16 = mybir.dt.int16
    AF = mybir.ActivationFunctionType
    ALU = mybir.AluOpType
    AX = mybir.AxisListType.X

    bacc.get_activation_tables = _pinned_tables
    nc = bacc.Bacc("TRN2", target_bir_lowering=False, debug=False,
                   num_devices=NCORES)

    d_hT = nc.dram_tensor("hT", [HID, NT], BF16, kind="ExternalInput")
    d_w = {k: nc.dram_tensor(k, [HID, 128], BF16, kind="ExternalInput")
           for k in ("wq", "wk", "wv")}
    d_b = {k: nc.dram_tensor(k, [128, 1], F32, kind="ExternalInput")
           for k in ("bq", "bk", "bv")}
    d_wc = nc.dram_tensor("wc", [HID, HID - 1], BF16, kind="ExternalInput")
    d_bc = nc.dram_tensor("bc", [1, HID - 1], F32, kind="ExternalInput")
    d_cos = nc.dram_tensor("cosA", [128, S], BF16, kind="ExternalInput")
    d_sin = nc.dram_tensor("sinA", [128, S], F32, kind="ExternalInput")
    d_pswap = nc.dram_tensor("pswap", [128, 128], BF16, kind="ExternalInput")
    d_ident = nc.dram_tensor("ident", [128, 128], BF16, kind="ExternalInput")
    d_ones2 = nc.dram_tensor("ones2", [128, 2], BF16, kind="ExternalInput")
    d_sgn2 = nc.dram_tensor("sgn2", [128, 2], BF16, kind="ExternalInput")
    d_sel2 = nc.dram_tensor("sel2", [2, 128], F32, kind="ExternalInput")
    d_out = nc.dram_tensor("out", [NT // NCORES, HID], F32, kind="ExternalOutput")

    with TileCtx(nc, tile) as (tc, ctx):
        sb = lambda name, bufs: ctx.enter_context(tc.tile_pool(name=name, bufs=bufs))
        consts = sb("consts", 1)
        ht_pool = sb("ht", 8)
        slabs = sb("slabs", 2)
        scratch = sb("scratch", 2)
        pt_pool = sb("pt", 4)
        nrm_pool = sb("nrm", 2)
        outs_pool = sb("outs", 2)
        ats_pool = sb("ats", 2)
        pp = ctx.enter_context(tc.tile_pool(name="pp", bufs=2, space="PSUM"))
        sc_ps = ctx.enter_context(tc.tile_pool(name="sc_ps", bufs=2, space="PSUM"))
        u_ps = ctx.enter_context(tc.tile_pool(name="u_ps", bufs=1, space="PSUM"))
        dram = ctx.enter_context(tc.tile_pool(name="dram", bufs=1, space="DRAM"))

        # ---- constants ----
        w_sb = {}
        for k in ("wq", "wk", "wv"):
            t = consts.tile([128, HID], BF16, name=f"{k}_sb")
            for kk in range(8):
                nc.sync.dma_start(t[:, 128 * kk:128 * (kk + 1)],
                                  d_w[k][128 * kk:128 * (kk + 1), :])
            w_sb[k] = t
        b_sb = {}
        for k in ("bq", "bk", "bv"):
            t = consts.tile([128, 1], F32, name=f"{k}_sb")
            nc.sync.dma_start(t[:], d_b[k][:])
            b_sb[k] = t
        NO = HID - 1  # 1023
        wc_sb = consts.tile([128, 8 * NO], BF16, name="wc_sb")
        for kk in range(8):
            nc.sync.dma_start(wc_sb[:, NO * kk:NO * (kk + 1)],
                              d_wc[128 * kk:128 * (kk + 1), :])
        bc_sb = consts.tile([1, NO], F32, name="bc_sb")
        nc.sync.dma_start(bc_sb[:], d_bc[:])
        cos_sb = consts.tile([128, S], BF16, name="cos_sb")
        nc.sync.dma_start(cos_sb[:], d_cos[:])
        sin_sb = consts.tile([128, S], F32, name="sin_sb")
        nc.sync.dma_start(sin_sb[:], d_sin[:])
        pswap_sb = consts.tile([128, 128], BF16, name="pswap_sb")
        nc.sync.dma_start(pswap_sb[:], d_pswap[:])
        ident_sb = consts.tile([128, 128], BF16, name="ident_sb")
        nc.sync.dma_start(ident_sb[:], d_ident[:])
        ones2 = consts.tile([128, 2], BF16, name="ones2")
        nc.sync.dma_start(ones2[:], d_ones2[:])
        sgn2 = consts.tile([128, 2], BF16, name="sgn2")
        nc.sync.dma_start(sgn2[:], d_sgn2[:])
        sel2 = consts.tile([2, 128], F32, name="sel2")
        nc.sync.dma_start(sel2[:], d_sel2[:])
        onesrow = consts.tile([1, 128], F32, name="onesrow")
        nc.vector.memset(onesrow[:], 1.0)
        ccb = consts.tile([128, 1], F32, name="ccb")
        nc.vector.memset(ccb[:], cc)

        a2a_in = [dram.tile([NCORES, 128, 256], BF16, name=f"a2a_in{b}",
                            tag=f"a2a_in{b}") for b in range(B)]
        a2a_out = [dram.tile([NCORES, 128, 256], BF16, name=f"a2a_out{b}",
                             tag=f"a2a_out{b}") for b in range(B)]

        slab_of = {}  # b -> (qf, k, v)
        hts_of = {}   # b -> list of 8 [128, S] tiles

        def load_hts(b):
            hts = []
            for kk in range(8):
                htk = ht_pool.tile([128, S], BF16, name="htk", tag="ht")
                nc.sync.dma_start(htk[:], d_hT[128 * kk:128 * (kk + 1),
                                              S * b:S * (b + 1)])
                hts.append(htk)
            hts_of[b] = hts

        def proj_block(b, blk):
            """Projection + rope + lift for tokens [S*b + BLK*blk, +BLK)."""
            qf_slab, k_slab, v_slab = slab_of[b]
            cols = slice(BLK * blk, BLK * (blk + 1))
            hts = [t[:, cols] for t in hts_of[b]]

            ss = pp.tile([128, BLK], F32, name="ss_ps", tag="pp")
            for iq, (name, slab) in enumerate((("q", qf_slab), ("k", k_slab))):
                ps = pp.tile([128, BLK], F32, name="proj_ps", tag="pp")
                for kk in range(8):
                    nc.tensor.matmul(ps[:],
                                     w_sb["w" + name][:, 128 * kk:128 * (kk + 1)],
                                     hts[kk],
                                     start=(kk == 0), stop=(kk == 7))
                x_bf = scratch.tile([128, BLK], BF16, name="x_bf", tag="x")
                nc.vector.tensor_scalar_add(x_bf[:], ps[:], b_sb["b" + name][:])
                # lift sum-of-squares from PRE-rope x (rotation invariant);
                # q rows -> ss[0:2], k rows -> ss[32:34] (col-tiled)
                sq = scratch.tile([128, BLK], BF16, name="sq", tag="sq")
                nc.vector.tensor_tensor(sq[:], x_bf[:], x_bf[:], ALU.mult)
                nc.tensor.matmul(ss[32 * iq:32 * iq + 2, :], ones2[:], sq[:],
                                 start=True, stop=True,
                                 tile_position=(0, 32 * iq))
                # rope
                swp = pp.tile([128, BLK], F32, name="swp_ps", tag="pp")
                nc.tensor.matmul(swp[:], pswap_sb[:], x_bf[:],
                                 start=True, stop=True)
                r1 = scratch.tile([128, BLK], BF16, name="r1", tag="r1")
                nc.vector.tensor_tensor(r1[:], x_bf[:], cos_sb[:, cols], ALU.mult)
                r2 = scratch.tile([128, BLK], BF16, name="r2", tag="r2")
                nc.vector.tensor_tensor(r2[:], swp[:], sin_sb[:, cols], ALU.mult)
                nc.vector.tensor_tensor(slab[:, cols], r1[:], r2[:], ALU.add)
            # q+k time rows: one Ln+Exp over the combined ss tile
            texp = scratch.tile([128, BLK], BF16, name="texp", tag="texp")
            lns = scratch.tile([128, BLK], F32, name="lns", tag="lns")
            nc.scalar.activation(lns[0:34, :], ss[0:34, :], AF.Ln, bias=ccb[0:34, :])
            nc.scalar.activation(texp[0:34, :], lns[0:34, :], AF.Exp, scale=0.5)
            tq = scratch.tile([2, BLK], BF16, name="tq", tag="tq")
            nc.vector.tensor_scalar_mul(tq[:], texp[0:2, :], -1.0)
            qrows = qf_slab[:].rearrange("(a c) n -> a c n", a=2)[:, 0, cols]
            nc.sync.dma_start(qrows, tq[:])
            krows = k_slab[:].rearrange("(a c) n -> a c n", a=2)[:, 0, cols]
            nc.sync.dma_start(krows, texp[32:34, :])

            # V: feature-major proj, transpose to token-major, lift post-hoc
            ps = pp.tile([128, BLK], F32, name="proj_ps", tag="pp")
            for kk in range(8):
                nc.tensor.matmul(ps[:],
                                 w_sb["wv"][:, 128 * kk:128 * (kk + 1)],
                                 hts[kk][:],
                                 start=(kk == 0), stop=(kk == 7))
            v_bf = scratch.tile([128, BLK], BF16, name="v_bf", tag="x")
            nc.vector.tensor_scalar_add(v_bf[:], ps[:], b_sb["bv"][:])
            tr = pp.tile([128, BLK], BF16, name="tr_ps", tag="pp")
            for j in range(4):
                nc.tensor.transpose(tr[:, 128 * j:128 * (j + 1)],
                                    v_bf[:, 128 * j:128 * (j + 1)], ident_sb[:])
            nc.vector.tensor_copy(v_slab[:, cols], tr[:])
            vview = v_slab[:, cols].rearrange("p (a h d) -> p a h d", a=4, h=2)
            sqv = scratch.tile([128, 4 * 2 * 63], F32, name="sqv", tag="sqv")
            sqvv = sqv[:].rearrange("p (a h d) -> p a h d", a=4, h=2)
            nc.vector.tensor_tensor(sqvv, vview[:, :, :, 1:64],
                                    vview[:, :, :, 1:64], ALU.mult)
            rv = scratch.tile([128, 8], F32, name="rv", tag="rv")
            nc.vector.tensor_reduce(rv[:].rearrange("p (a h) -> p a h", a=4),
                                    sqvv, AX, ALU.add)
            lnv = scratch.tile([128, 8], F32, name="lnv", tag="lnv")
            nc.scalar.activation(lnv[:], rv[:], AF.Ln, bias=ccb[:])
            tv = scratch.tile([128, 8], F32, name="tv", tag="tv")
            nc.scalar.activation(tv[:], lnv[:], AF.Exp, scale=0.5)
            nc.vector.tensor_copy(vview[:, :, :, 0:1],
                                  tv[:].rearrange("p (a h) -> p a h", a=4).unsqueeze(3))

        def attn_steps(b, qc, interleave):
            """32 (h, kt) steps + renorm + att store for query chunk qc."""
            qf_slab, k_slab, v_slab = slab_of[b]
            u = u_ps.tile([128, QC], F32, name="u_ps", tag="u")
            for s in range(32):
                if s in interleave:
                    interleave[s]()
                h = s % 2
                kt = s // 2
                hp = slice(64 * h, 64 * h + 64)
                sc = sc_ps.tile([128, QC], F32, name="sc_ps", tag="sc")
                for half in range(2):
                    nc.tensor.matmul(
                        sc[:, 512 * half:512 * (half + 1)],
                        k_slab[hp, 128 * kt:128 * (kt + 1)],
                        qf_slab[hp, QC * qc + 512 * half:QC * qc + 512 * (half + 1)],
                        start=True, stop=True, tile_position=(64 * h, 0))
                if DVE_STEP(h, kt):
                    pt_i = pt_pool.tile([128, QC], I16, name="pt_i", tag="pt_i")
                    nc.vector.tensor_scalar(pt_i[:], sc[:], SCHR_A, SCHR_B,
                                            ALU.mult, ALU.add)
                    pt = pt_i[:].bitcast(BF16)
                else:
                    pt_t = pt_pool.tile([128, QC], BF16, name="pt", tag="pt")
                    nc.scalar.activation(pt_t[:], sc[:], AF.Exp, scale=SCALE)
                    pt = pt_t[:]
                vt = v_slab[:, 128 * kt + 64 * h:128 * kt + 64 * h + 64]
                for half in range(2):
                    nc.tensor.matmul(
                        u[hp, 512 * half:512 * (half + 1)],
                        vt,
                        pt[:, 512 * half:512 * (half + 1)],
                        start=(kt == 0), stop=(kt == NKT - 1),
                        tile_position=(0, 64 * h))
            # Lorentz renormalize both heads at once (denominator cancels)
            ucp = nrm_pool.tile([128, QC], BF16, name="ucp", tag="ucp")
            nc.vector.tensor_copy(ucp[:], u[:])
            nsq = nrm_pool.tile([128, QC], BF16, name="nsq", tag="nsq")
            nc.vector.tensor_tensor(nsq[:], ucp[:], ucp[:], ALU.mult)
            for half in range(2):
                hs = slice(512 * half, 512 * (half + 1))
                lp = pp.tile([2, 512], F32, name="lp_ps", tag="pp")
                nc.tensor.matmul(lp[:], sgn2[:], nsq[:, hs], start=True, stop=True)
                lg = nrm_pool.tile([2, 512], F32, name="lg", tag="lg")
                nc.scalar.activation(lg[:], lp[:], AF.Ln, scale=-1.0)
                rr = nrm_pool.tile([2, 512], F32, name="rr", tag="rr")
                nc.scalar.activation(rr[:], lg[:], AF.Exp, scale=-0.5)
                rb = pp.tile([128, 512], F32, name="rb_ps", tag="pp")
                nc.tensor.matmul(rb[:], sel2[:], rr[:], start=True, stop=True)
                att = nrm_pool.tile([128, 512], BF16, name="att", tag="att")
                nc.vector.tensor_tensor(att[:], ucp[:, hs], rb[:], ALU.mult)
                for cx in range(2):
                    j = 4 * qc + 2 * half + cx
                    nc.sync.dma_start(a2a_in[b][j, :, :],
                                      att[:, 256 * cx:256 * (cx + 1)])

        def do_a2a(b):
            nc.gpsimd.collective_compute(
                "AllToAll", mybir.AluOpType.bypass,
                replica_groups=[list(range(NCORES))],
                ins=[a2a_in[b][:].opt()],
                outs=[a2a_out[b][:].opt()],
            )

        atts_of = {}

        def load_atts(b):
            t = ats_pool.tile([128, 8 * 256], BF16, name="attk", tag="attk")
            nc.sync.dma_start(t[:].rearrange("p (k c) -> p k c", k=8),
                              a2a_out[b][:].rearrange("k p c -> p k c"))
            atts_of[b] = t

        def outproj_m(b, m):
            """Output projection for 128 tokens (m-th tile of batch b's slice)."""
            att_k = atts_of[b]
            msl = slice(128 * m, 128 * (m + 1))
            pss = []
            for n0 in (0, NO - 512):  # second half overlaps one col
                po = pp.tile([128, 512], F32, name="out_ps", tag="pp")
                for kk in range(8):
                    nc.tensor.matmul(po[:],
                                     att_k[:, 256 * kk + msl.start:
                                           256 * kk + msl.stop],
                                     wc_sb[:, NO * kk + n0:NO * kk + n0 + 512],
                                     start=(kk == 0), stop=False)
                nc.tensor.matmul(po[:], onesrow[:], bc_sb[:, n0:n0 + 512],
                                 start=False, stop=True)
                pss.append(po)
            out_sb = outs_pool.tile([128, HID], F32, name="out_sb", tag="out")
            nc.scalar.activation(out_sb[:, 1:513], pss[0][:], AF.Copy)
            nc.scalar.activation(out_sb[:, 513:HID], pss[1][:, 1:512], AF.Copy)
            sqo = scratch.tile([128, NO], BF16, name="sqo", tag="sqo")
            nc.vector.tensor_tensor(sqo[:], out_sb[:, 1:HID], out_sb[:, 1:HID],
                                    ALU.mult)
            ro = scratch.tile([128, 1], F32, name="ro", tag="ro")
            nc.vector.tensor_reduce(ro[:], sqo[:].unsqueeze(1), AX, ALU.add)
            lno = scratch.tile([128, 1], F32, name="lno", tag="lno")
            nc.scalar.activation(lno[:], ro[:], AF.Ln, bias=ccb[:])
            nc.scalar.activation(out_sb[:, 0:1], lno[:], AF.Exp, scale=0.5)
            nc.sync.dma_start(d_out[256 * b + msl.start:256 * b + msl.stop, :],
                              out_sb[:])

        # ---------------- emission schedule ----------------
        def new_slabs(b):
            slab_of[b] = (
                slabs.tile([128, S], BF16, name="qf_slab", tag="qf"),
                slabs.tile([128, S], BF16, name="k_slab", tag="kk"),
                slabs.tile([128, S], BF16, name="v_slab", tag="vv"),
            )

        new_slabs(0)
        load_hts(0)
        for blk in range(NB):
            proj_block(0, blk)

        for b in range(B):
            if b > 0:
                do_a2a(b - 1)
                load_atts(b - 1)
            # qc=0: interleave next batch's projection blocks
            il0 = {}
            if b + 1 < B:
                new_slabs(b + 1)
                il0 = {0: lambda bb=b: load_hts(bb + 1),
                       2: lambda bb=b: proj_block(bb + 1, 0),
                       10: lambda bb=b: proj_block(bb + 1, 1),
                       18: lambda bb=b: proj_block(bb + 1, 2),
                       26: lambda bb=b: proj_block(bb + 1, 3)}
            attn_steps(b, 0, il0)
            # qc=1: interleave previous batch's output projection
            il1 = {}
            if b > 0:
                il1 = {8: lambda bb=b: outproj_m(bb - 1, 0),
                       20: lambda bb=b: outproj_m(bb - 1, 1)}
            attn_steps(b, 1, il1)

        do_a2a(B - 1)
        load_atts(B - 1)
        outproj_m(B - 1, 0)
        outproj_m(B - 1, 1)

    try:
        nc.compile()
    finally:
        bacc.get_activation_tables = _orig_tables
    return nc


class TileCtx:
    """TileContext + an ExitStack for pools, as one context manager."""

    def __init__(self, nc, tile_mod):
        self.nc = nc
        self.tile_mod = tile_mod

    def __enter__(self):
        self.ctx = ExitStack()
        self.tc = self.tile_mod.TileContext(self.nc)
        self.tc.__enter__()
        return self.tc, self.ctx

    def __exit__(self, *exc):
        self.ctx.close()
        return self.tc.__exit__(*exc)


def _get_graph(cc):
    key = round(cc, 9)
    if key not in _GRAPH_CACHE:
        _GRAPH_CACHE[key] = _build(cc)
    return _GRAPH_CACHE[key]


def _gather(results):
    """results[j]['out'] is [B*256, HID] covering, for each batch, tokens
    [qc*1024 + chunk*256, +256) with qc=j//4, chunk=j%4."""
    out = np.empty((B, S, HID), np.float32)
    for j in range(NCORES):
        r = np.asarray(results[j]["out"], np.float32).reshape(B, 256, HID)
        s0 = (j // 4) * 1024 + (j % 4) * 256
        out[:, s0:s0 + 256, :] = r
    return out


def run(trace=False, **inputs):
    from concourse.bass_utils import run_bass_kernel_spmd

    in_maps, cc = _host_prep(**inputs)
    nc = _get_graph(cc)
    res = run_bass_kernel_spmd(nc, in_maps, list(range(NCORES)), trace=trace)
    out = _gather(res.results)
    return out, res


def kernel(**inputs):
    out, _ = run(trace=False, **inputs)
    return out
